# revision 2
# baseline (speedup 1.0000x reference)
# Trainium2 Bass kernel for LinearAttention (nn_LinearAttention_87686052315975).
#
# Reference computation (per batch element b of 16):
#   xf = x[b].reshape(512, 4096)                      # [c, l]
#   qkv = w_qkv @ xf; q, k, v split into 8 heads x 64 dims
#   k = softmax(k, axis=l)
#   context_h = k_h @ v_h^T                           # [64, 64]
#   out_h = context_h^T @ q_h                          # [64, l]
#   y = w_out @ concat(out_h) + b_out                 # [512, l]
#
# Restructure: fold the tiny per-head context into the weights.
#   y = sum_h Wout_h ctxn_h^T Wq_h x = M x  with M [512, 512].
# So: pass 1 computes k/v projections transposed (l on partitions) and
# accumulates ctx = E @ [v|1]^T in PSUM across all l (ones columns give the
# softmax denominator for free); finalize normalizes ctx; build-M turns it
# into M^T; pass 2 is one plain matmul y = M x + bias.
#
# vs the previous version: everything runs bf16 on the PE (x, weights cast
# to bf16 on the HOST, which also pre-tiles every tensor so each DMA is a
# single contiguous block with 4-8KB per-partition segments).  This removes
# all on-chip f32->bf16 casts, halves x DMA bytes, avoids fp32<->bf16 PE
# mode switches (bf16 sustains ~216ns per N=512 matmul vs ~232ns fp32r),
# and frees the scalar engine for exp only.  Phases run sequentially per
# batch (p1(0), M(0), p2(0), p1(1), M(1), p2(1)) so the 16MB of y writes
# spread across the kernel instead of bunching at the end.  y stores issue
# from the scalar queue (Activation is also a HWDGE engine) so they never
# serialize behind x loads on the sync queue.

import numpy as np
from contextlib import ExitStack

import ml_dtypes

import concourse.bass as bass
import concourse.bacc as bacc
import concourse.mybir as mybir
import concourse.tile as tile

# ---- problem constants (hardcoded per contract) ----
B, DIM, HGT, WID = 16, 512, 64, 64
L = HGT * WID            # 4096
HEADS, DH = 8, 64
HIDDEN = HEADS * DH      # 512
NCORES = 8
BPC = B // NCORES        # 2 batches per core
P = 128
CHUNK = 512
NCHUNK = L // CHUNK      # 8
KT = DIM // P            # 4 contraction tiles over channels
MT = DIM // P            # 4 output row tiles
LM = CHUNK // P          # 4 l-subtiles per chunk
NPAIR = HEADS // 2       # 4 head pairs
VW = DH + 2              # per-head vT width: 64 v cols + 2 ones cols
CTXW = 2 * VW            # 132: one pair's context block width

F32 = mybir.dt.float32
BF16 = mybir.dt.bfloat16
BF16NP = ml_dtypes.bfloat16


def build_kernel(ctx: ExitStack, tc: "tile.TileContext", x_in, wkvk_in,
                 wkvv_in, wq_in, wout_in, bias_in, y_out):
    nc = tc.nc

    wpool = ctx.enter_context(tc.tile_pool(name="weights", bufs=1))
    xpool = ctx.enter_context(tc.tile_pool(name="xres", bufs=1))
    epool = ctx.enter_context(tc.tile_pool(name="ev", bufs=8))
    vpool = ctx.enter_context(tc.tile_pool(name="vt", bufs=8))
    cpool = ctx.enter_context(tc.tile_pool(name="ctxp", bufs=8))
    apool = ctx.enter_context(tc.tile_pool(name="absf", bufs=4))
    mpool = ctx.enter_context(tc.tile_pool(name="mtbf", bufs=8))
    rpool = ctx.enter_context(tc.tile_pool(name="recip", bufs=8))
    ypool = ctx.enter_context(tc.tile_pool(name="ysb", bufs=4))
    psmm = ctx.enter_context(tc.tile_pool(name="psmm", bufs=4, space="PSUM"))
    psctx = ctx.enter_context(tc.tile_pool(name="psctx", bufs=2, space="PSUM"))
    ps2 = ctx.enter_context(tc.tile_pool(name="ps2", bufs=2, space="PSUM"))

    # ---- persistent SBUF tensors ----
    # x resident, bf16, one tile per batch: col = k*L + i*CHUNK + l
    xt = [xpool.tile([P, KT * L], BF16, tag=f"x{b}", name=f"x{b}")
          for b in range(BPC)]
    # wkv: col = k*1024 + half*512 + c   (half 0 = k-proj, 1 = v-proj)
    wkv_sb = wpool.tile([P, KT * 2 * HIDDEN], BF16, tag="wkv", name="wkv")
    wq_bf = wpool.tile([P, NPAIR * DIM], BF16, tag="wq", name="wq")
    wout_bf = wpool.tile([P, NPAIR * DIM], BF16, tag="wout", name="wout")
    bias_sb = wpool.tile([P, MT], F32, tag="bias", name="bias")

    def xslice(b, k, lo, hi):
        return xt[b][:, k * L + lo:k * L + hi]

    def wkvslice(k, half):
        base = k * 2 * HIDDEN + half * HIDDEN
        return wkv_sb[:, base:base + HIDDEN]

    wkv_view = wkv_sb[:].rearrange("p (k w) -> p k w", w=2 * HIDDEN)

    # ---- startup DMAs, all on the sync queue (the scalar HWDGE queue has
    # ~3us first-packet latency — it only carries latency-tolerant y
    # stores).  Order matches first-consumption: the first matmul needs
    # x(b0,c0,k0/1) + wkv k-half.
    xv0 = xt[0][:].rearrange("p (k l) -> p k l", k=KT)
    nc.sync.dma_start(xv0[:, 0:2, 0:CHUNK], x_in[0, 0, :, 0:2, :])
    nc.sync.dma_start(wkv_view[:, :, 0:HIDDEN], wkvk_in[:, :, :])
    nc.sync.dma_start(xv0[:, 2:4, 0:CHUNK], x_in[0, 0, :, 2:4, :])
    nc.sync.dma_start(wkv_view[:, :, HIDDEN:2 * HIDDEN], wkvv_in[:, :, :])
    nc.sync.dma_start(bias_sb[:], bias_in[:])
    nc.sync.dma_start(wq_bf[:], wq_in[:])
    nc.sync.dma_start(wout_bf[:], wout_in[:])
    # PE p-state warm-up: dummy matmuls bridge the ENTIRE initial DMA wait
    # (~6us) so the DVFS ramp (0.65->2.4GHz after ~3us of continuous busy)
    # completes — and doesn't decay in an idle gap — before the first real
    # matmul.  Scratch memset on gpsimd, whose queue starts earliest.
    scratch = wpool.tile([P, CHUNK], BF16, tag="warm", name="warm")
    nc.gpsimd.memset(scratch[:], 0.0)
    warm_ps = ps2.tile([P, CHUNK], F32, tag="mm2", name="mm2")
    for _ in range(8):
        nc.tensor.matmul(warm_ps[:], scratch[:, 0:P], scratch[:],
                         start=True, stop=True)
    # pre-zeroed block-diag ctx tiles (8 = 2 batches x 4 pairs); finalize
    # only writes the diagonal blocks, so the memsets run once, off the
    # critical path, on the otherwise-idle gpsimd engine.
    ctxP_all = []
    for t in range(2 * NPAIR):
        ct_t = cpool.tile([P, P], BF16, tag="ctxP", name="ctxP")
        nc.gpsimd.memset(ct_t[:], 0.0)
        ctxP_all.append(ct_t)
    # persistent vT tiles: the two ones-columns per head are written once
    # here and never touched again — pass 1 only rewrites the v columns.
    vT_all = []
    for t in range(8):
        vt_t = vpool.tile([P, HEADS * VW], BF16, tag="vT", name="vT")
        nc.vector.memset(
            vt_t[:].rearrange("p (h e) -> p h e", e=VW)[:, :, DH:VW], 1.0)
        vT_all.append(vt_t)
    # remaining x chunks for both batches; sync queue has nothing else to do
    # until the y stores (which live on the scalar queue anyway).
    for b in range(BPC):
        for i in range(NCHUNK):
            if b == 0 and i == 0:
                continue
            nc.sync.dma_start(
                xt[b][:].rearrange("p (k l) -> p k l", k=KT)[
                    :, :, i * CHUNK:(i + 1) * CHUNK],
                x_in[b, i, :, :, :])

    ctxP = {}      # batch -> 4 block-diag bf16 [128, 128] normalized ctx
    ctx_ps = {}    # batch -> 2 PSUM tiles [128, 264] (2 pairs each)

    def pass1(b):
        ctx_ps[b] = [psctx.tile([P, 2 * CTXW], F32, tag="ctx", name="ctx")
                     for _ in range(2)]
        for i in range(NCHUNK):
            E_t, vT_t = [], []

            def kproj(lm):
                lo = i * CHUNK + lm * P
                # kT: [128 l, 512 (h,d)] -> E = exp
                ps = psmm.tile([P, HIDDEN], F32, tag="mm", name="mm")
                for k in range(KT):
                    nc.tensor.matmul(ps[:], xslice(b, k, lo, lo + P),
                                     wkvslice(k, 0),
                                     start=(k == 0), stop=(k == KT - 1))
                e = epool.tile([P, HIDDEN], BF16, tag="E", name="E")
                nc.scalar.activation(e[:], ps[:],
                                     mybir.ActivationFunctionType.Exp)
                E_t.append(e)

            def vproj(lm):
                lo = i * CHUNK + lm * P
                # vT: [128 l, 512 (h,e)] -> bf16 with ones cols per head
                ps = psmm.tile([P, HIDDEN], F32, tag="mm", name="mm")
                for k in range(KT):
                    nc.tensor.matmul(ps[:], xslice(b, k, lo, lo + P),
                                     wkvslice(k, 1),
                                     start=(k == 0), stop=(k == KT - 1))
                v = vT_all[(i * LM + lm) % 8]
                v_view = v[:].rearrange("p (h e) -> p h e", e=VW)
                nc.vector.tensor_copy(
                    v_view[:, :, 0:DH],
                    ps[:].rearrange("p (h e) -> p h e", e=DH))
                vT_t.append(v)

            if b == 0 and i == 0:
                # first chunk: all k-projections first — 16 matmuls of real
                # work fill the window before the v-weights DMA lands
                for lm in range(LM):
                    kproj(lm)
                for lm in range(LM):
                    vproj(lm)
            else:
                for lm in range(LM):
                    kproj(lm)
                    vproj(lm)

            # context accumulation into persistent PSUM, one matmul per
            # head pair (block-diag packing; off-diag blocks never read).
            # start=True resets the WHOLE psum bank, so only the first
            # pair sharing a bank may issue it; the second pair
            # accumulates from zero.
            for lm in range(LM):
                for p in range(NPAIR):
                    reg = ctx_ps[b][p // 2][:, (p % 2) * CTXW:
                                            (p % 2 + 1) * CTXW]
                    nc.tensor.matmul(
                        reg,
                        E_t[lm][:, p * P:(p + 1) * P],
                        vT_t[lm][:, p * CTXW:(p + 1) * CTXW],
                        start=(i == 0 and lm == 0 and p % 2 == 0),
                        stop=(i == NCHUNK - 1 and lm == LM - 1),
                        skip_group_check=(p % 2 == 1))

    def finalize(b):
        # normalize ctx rows by the accumulated rowsum -> block-diag bf16.
        # recips on vector (vector-only op); the scale-multiplies split
        # vector / gpsimd so pair tiles are produced ~2x faster and the
        # tensor engine's A-matmuls don't starve.
        ctxP[b] = []
        for p in range(NPAIR):
            acc = ctx_ps[b][p // 2]
            base = (p % 2) * CTXW
            r = rpool.tile([P, 1], F32, tag="recip", name="recip")
            nc.vector.reciprocal(r[0:DH, 0:1],
                                 acc[0:DH, base + DH:base + DH + 1])
            nc.vector.reciprocal(r[DH:P, 0:1],
                                 acc[DH:P, base + CTXW - 2:base + CTXW - 1])
            t = ctxP_all[b * NPAIR + p]
            # per pair, one diag-block scale on vector and one on scalar
            # (gpsimd cannot read PSUM) — halves the produce latency
            nc.vector.tensor_scalar_mul(t[0:DH, 0:DH],
                                        acc[0:DH, base:base + DH],
                                        r[0:DH, 0:1])
            nc.scalar.mul(t[DH:P, DH:P],
                          acc[DH:P, base + VW:base + VW + DH],
                          r[DH:P, 0:1])
            ctxP[b].append(t)

    def build_m(b):
        # A_pair = ctxn_pair^T @ Wq_pair : [128 (h,e), 512 c], then
        # M^T[c, o] = sum_pairs A_pair[he, c]^T-contract WoutT_pair[he, o].
        # The Mt accumulations interleave with the A matmuls per pair so the
        # tensor engine keeps working while finalize/copies pipeline; psum
        # ring: A0,A1 -> psmm b0,b1; Mt2,Mt3 -> psmm b2,b3; A2,A3 wrap to
        # b0,b1 after the A0/A1 copies drain.  Copies alternate
        # vector/scalar so the serial chain halves.
        def a_matmul(p):
            ps = psmm.tile([P, DIM], F32, tag="mm", name="mm")
            nc.tensor.matmul(ps[:], ctxP[b][p][:],
                             wq_bf[:, p * DIM:(p + 1) * DIM],
                             start=True, stop=True)
            a = apool.tile([P, DIM], BF16, tag="A", name="A")
            # scalar's ACTIVATE copy (~0.4us) beats vector's CAST (~0.7us)
            nc.scalar.copy(a[:], ps[:])
            return a

        A_bf = {p: a_matmul(p) for p in (0, 1)}
        Mt_ps = [ps2.tile([P, DIM], F32, tag="mm2", name="mm2") if ct < 2
                 else psmm.tile([P, DIM], F32, tag="mm", name="mm")
                 for ct in range(KT)]

        Mt_bf = [mpool.tile([P, DIM], BF16, tag="Mt", name="Mt")
                 for _ in range(KT)]

        def mt_partials(p):
            for ct in range(KT):
                nc.tensor.matmul(Mt_ps[ct][:],
                                 A_bf[p][:, ct * P:(ct + 1) * P],
                                 wout_bf[:, p * DIM:(p + 1) * DIM],
                                 start=(p == 0), stop=(p == NPAIR - 1))
                if p == NPAIR - 1:
                    # copy each Mt tile out the moment its accumulation
                    # finishes, so pass 2 isn't gated on the last copy
                    if ct % 2 == 0:
                        nc.vector.tensor_copy(Mt_bf[ct][:], Mt_ps[ct][:])
                    else:
                        nc.scalar.copy(Mt_bf[ct][:], Mt_ps[ct][:])

        mt_partials(0)
        mt_partials(1)
        for p in (2, 3):
            A_bf[p] = a_matmul(p)
            mt_partials(p)
        return Mt_bf

    def pass2(b, Mt_bf):
        # y = M x + bias.  bias-adds for m0/m1 on vector, m2/m3 on scalar;
        # the chunk's store splits into an m01 half issued from sync (idle
        # after the x prefetch) and an m23 half from scalar, so each store
        # starts as soon as its own adds drain and the two DMA queues share
        # the write bandwidth.
        for i in range(NCHUNK):
            last = (b == BPC - 1 and i == NCHUNK - 1)
            yw = ypool.tile([P, MT * CHUNK], BF16, tag="ysb", name="ysb")
            for m in range(MT):
                if m < 2:
                    ps = psmm.tile([P, CHUNK], F32, tag="mm", name="mm")
                else:
                    ps = ps2.tile([P, CHUNK], F32, tag="mm2", name="mm2")
                lo = i * CHUNK
                for ct in range(KT):
                    nc.tensor.matmul(
                        ps[:],
                        Mt_bf[ct][:, m * P:(m + 1) * P],
                        xslice(b, ct, lo, lo + CHUNK),
                        start=(ct == 0), stop=(ct == KT - 1))
                yv = yw[:, m * CHUNK:(m + 1) * CHUNK]
                # final chunk: strictly alternate add engines so the last
                # add (m3) starts the moment its matmuls finish
                if (m % 2 == 0) if last else (m < 2):
                    nc.vector.tensor_scalar_add(yv, ps[:],
                                                bias_sb[:, m:m + 1])
                else:
                    nc.scalar.add(yv, ps[:], bias_sb[:, m:m + 1])
                if last:
                    # per-m stores on alternating queues; the very last
                    # m-tile splits across BOTH queues (64KB each) so the
                    # end-of-kernel drain is as short as possible
                    if m == MT - 1:
                        half = CHUNK // 2
                        nc.sync.dma_start(
                            y_out[b, i][:, m:m + 1, 0:half],
                            yv[:, 0:half].rearrange("p (m l) -> p m l", m=1))
                        nc.scalar.dma_start(
                            y_out[b, i][:, m:m + 1, half:CHUNK],
                            yv[:, half:CHUNK].rearrange("p (m l) -> p m l",
                                                        m=1))
                    else:
                        eng = nc.sync if m % 2 == 0 else nc.scalar
                        eng.dma_start(
                            y_out[b, i][:, m:m + 1, :],
                            yv.rearrange("p (m l) -> p m l", m=1))
            if not last:
                nc.sync.dma_start(
                    y_out[b, i][:, 0:2, :],
                    yw[:, 0:2 * CHUNK].rearrange("p (m l) -> p m l", m=2))
                nc.scalar.dma_start(
                    y_out[b, i][:, 2:4, :],
                    yw[:, 2 * CHUNK:4 * CHUNK].rearrange("p (m l) -> p m l",
                                                         m=2))

    for b in range(BPC):
        pass1(b)
        finalize(b)
        Mt = build_m(b)
        pass2(b, Mt)


def build_module():
    nc = bacc.Bacc("TRN2", target_bir_lowering=False, debug=False,
                   num_devices=NCORES)
    x_in = nc.dram_tensor("x", [BPC, NCHUNK, P, KT, CHUNK], BF16,
                          kind="ExternalInput")
    wkvk_in = nc.dram_tensor("w_kvk", [P, KT, HIDDEN], BF16,
                             kind="ExternalInput")
    wkvv_in = nc.dram_tensor("w_kvv", [P, KT, HIDDEN], BF16,
                             kind="ExternalInput")
    wq_in = nc.dram_tensor("w_q", [P, NPAIR * DIM], BF16,
                           kind="ExternalInput")
    wout_in = nc.dram_tensor("w_outT", [P, NPAIR * DIM], BF16,
                             kind="ExternalInput")
    bias_in = nc.dram_tensor("bias", [P, MT], F32, kind="ExternalInput")
    y_out = nc.dram_tensor("y", [BPC, NCHUNK, P, MT, CHUNK], BF16,
                           kind="ExternalOutput")
    with tile.TileContext(nc) as tc:
        with ExitStack() as ctx:
            build_kernel(ctx, tc, x_in, wkvk_in, wkvv_in, wq_in, wout_in,
                         bias_in, y_out)
    nc.compile()
    return nc


def make_in_maps(x, w_qkv, w_out, b_out):
    x = np.ascontiguousarray(x, dtype=np.float32).reshape(B, DIM, L)
    # [B, DIM, L] -> [B, NCHUNK, P, KT, CHUNK] bf16, contiguous per chunk
    xt = x.reshape(B, KT, P, NCHUNK, CHUNK).transpose(0, 3, 2, 1, 4)
    xt = np.ascontiguousarray(xt.astype(BF16NP))
    w_qkv = np.asarray(w_qkv, dtype=np.float32)
    wkvT = w_qkv.T[:, HIDDEN:3 * HIDDEN]            # [512c, 1024 (k|v)]
    wkvk = np.ascontiguousarray(
        wkvT[:, 0:HIDDEN].reshape(KT, P, HIDDEN).transpose(1, 0, 2)
        .astype(BF16NP))
    wkvv = np.ascontiguousarray(
        wkvT[:, HIDDEN:2 * HIDDEN].reshape(KT, P, HIDDEN).transpose(1, 0, 2)
        .astype(BF16NP))
    wq = np.ascontiguousarray(
        w_qkv[0:HIDDEN, :].reshape(NPAIR, P, DIM).transpose(1, 0, 2)
        .reshape(P, NPAIR * DIM).astype(BF16NP))
    woutT = np.asarray(w_out, dtype=np.float32).T   # [512 he, 512 o]
    wout = np.ascontiguousarray(
        woutT.reshape(NPAIR, P, DIM).transpose(1, 0, 2)
        .reshape(P, NPAIR * DIM).astype(BF16NP))
    bias = np.ascontiguousarray(
        np.asarray(b_out, dtype=np.float32).reshape(MT, P).T)
    in_maps = []
    for c in range(NCORES):
        in_maps.append({
            "x": xt[c * BPC:(c + 1) * BPC],
            "w_kvk": wkvk,
            "w_kvv": wkvv,
            "w_q": wq,
            "w_outT": wout,
            "bias": bias,
        })
    return in_maps


_NC_CACHE = None


def kernel(x, w_qkv, w_out, b_out, *, trace=False, trace_kwargs=None):
    """Full inputs in, full output out. Shards batch across 8 NeuronCores."""
    global _NC_CACHE
    from concourse.bass_utils import run_bass_kernel_spmd

    if _NC_CACHE is None:
        _NC_CACHE = build_module()
    nc = _NC_CACHE

    in_maps = make_in_maps(x, w_qkv, w_out, b_out)
    kw = dict(trace_kwargs or {})
    res = run_bass_kernel_spmd(nc, in_maps, list(range(NCORES)),
                               trace=trace, **kw)
    y = np.empty((B, DIM, HGT, WID), dtype=np.float32)
    for c in range(NCORES):
        yd = np.asarray(res.results[c]["y"], dtype=np.float32)
        y[c * BPC:(c + 1) * BPC] = yd.transpose(0, 3, 2, 1, 4).reshape(
            BPC, DIM, HGT, WID)
    kernel.last_results = res
    return y
